# revision 1
# baseline (speedup 1.0000x reference)
"""Trainium2 Bass kernel for nn_AggregationLayer (per-class masked reductions + Hough voting).

Strategy (8 NeuronCores, data-parallel over batch: 2 samples/core):
  The device computes, per (class c in 1..6, sample b), 20 masked sums
      S_c[x] = sum_p [cat_p == c] * x_p
  over the 307200 pixels of each sample, for channels x in
      {1, q0..q3, s0..s2, z, dxh2, dyh2, m, dxh2*pu, m*pv, m*pu, dyh2*pv,
       puA, puB, pvA, pvB}
  where (dxh2, dyh2, m) = (dx^2, dy^2, dx*dy) / (|xy|^2 + delta) are the
  Hough direction-matrix terms and pu/pv are pixel column/row coordinates.
  pu is split as puA = 4*(pu//4), puB = pu%4 (both exactly representable in
  bf16, likewise pvA/pvB) so the position sums are exact integers in fp32.

  The segmented (per-class) reduction runs on the TensorEngine: for every
  128-pixel chunk (one column of the [128, 2400] plane layout), one
  self-loading bf16 matmul contracts the chunk: stationary = the chunk's 6
  one-hot columns, moving = its 20 channel values (strided access across the
  stacked channel planes), accumulating [6, 20] per-class sums in PSUM across
  all 2400 chunks of a sample. Elementwise channel builds run on DVE/ACT in
  parallel, 800-column slabs triple-buffered against the DMA loads.

  The host does only the tiny [6, B] finalization: 2x2 solve for the Hough
  center, quaternion -> rotation matrix, intrinsics backprojection, packing
  into the [6, 16, 26] output.
"""

import numpy as np
import ml_dtypes

B, H, W = 16, 480, 640
CLASSES = 7
C1 = CLASSES - 1
NCORES = 8
SPC = B // NCORES          # samples per core
NPART = 128
COLS = (H * W) // NPART    # 2400
SLAB = 800
NSLAB = COLS // SLAB       # 3
NCH = 20                   # moving channels
DELTA = 1e-12              # guard for 1/(n2 + DELTA)
EPS = 1e-6                 # matches reference

BF16 = ml_dtypes.bfloat16

# moving-channel slot map
S_ONE, S_Q, S_S, S_Z = 0, 1, 5, 8
S_DXH2, S_DYH2, S_M = 9, 10, 11
S_T1, S_T2, S_T3, S_T4 = 12, 13, 14, 15
S_PUA, S_PUB, S_PVA, S_PVB = 16, 17, 18, 19

_NC_CACHE = {}
_STATIC_CACHE = {}


def _build_static():
    if "st" in _STATIC_CACHE:
        return _STATIC_CACHE["st"]
    p = np.arange(H * W, dtype=np.int64)
    pu = (p % W).astype(np.float64)
    pv = (p // W).astype(np.float64)
    puA = (pu // 4) * 4.0
    puB = pu % 4
    pvA = (pv // 4) * 4.0
    pvB = pv % 4
    ones = np.ones_like(pu)

    def plane16(a):
        return a.reshape(NPART, COLS).astype(BF16)

    st16 = np.stack([plane16(ones), plane16(puA), plane16(puB),
                     plane16(pvA), plane16(pvB)])            # [5,128,2400] bf16
    st32 = np.stack([pu.reshape(NPART, COLS).astype(np.float32),
                     pv.reshape(NPART, COLS).astype(np.float32)])  # [2,128,2400] f32
    _STATIC_CACHE["st"] = (st16, st32)
    return st16, st32


def _build_nc(reps=1, feat_q="sync", dve_v2=True, cmp_gps=True, dma_split=1, wbufs=3, tbufs=3):
    """Build + compile the SPMD Bass program. reps > 1 wraps the whole
    pipeline in a hardware For loop (used only for benchmarking)."""
    key = (reps, feat_q, dve_v2, cmp_gps, dma_split, wbufs, tbufs, SLAB)
    if key in _NC_CACHE:
        return _NC_CACHE[key]
    import contextlib
    import concourse.bacc as bacc
    import concourse.mybir as mybir
    import concourse.tile as tile

    F32, MBF16 = mybir.dt.float32, mybir.dt.bfloat16
    AOT = mybir.AluOpType
    ACTF = mybir.ActivationFunctionType

    nc = bacc.Bacc("TRN2", target_bir_lowering=False, debug=False)
    feat_d = nc.dram_tensor("feat", [SPC, 8, NPART, COLS], MBF16, kind="ExternalInput")
    cat_d = nc.dram_tensor("cat", [SPC, NPART, COLS], MBF16, kind="ExternalInput")
    xy_d = nc.dram_tensor("xy", [SPC, 2, NPART, COLS], MBF16, kind="ExternalInput")
    st16_d = nc.dram_tensor("st16", [5, NPART, COLS], MBF16, kind="ExternalInput")
    st32_d = nc.dram_tensor("st32", [2, NPART, COLS], F32, kind="ExternalInput")
    sums_d = nc.dram_tensor("sums", [C1, SPC * NCH], F32, kind="ExternalOutput")

    with tile.TileContext(nc) as tc:
        with (
            tc.tile_pool(name="mov", bufs=1) as pmov,
            tc.tile_pool(name="stat", bufs=1) as pstat,
            tc.tile_pool(name="work", bufs=wbufs) as pwork,
            tc.tile_pool(name="tmp", bufs=tbufs) as ptmp,
            tc.tile_pool(name="psum", bufs=1, space="PSUM") as pps,
        ):
            # persistent moving buffers, one per slab phase (3-deep rotation);
            # static channel slots (ones/puA/puB/pvA/pvB) are written once per
            # physical buffer and survive the per-sample rewrites of slots 1-15
            m_bufs = []
            for k in range(NSLAB):
                mb = pmov.tile([NPART, NCH * SLAB], MBF16, name=f"Mbuf{k}", tag=f"Mbuf{k}")
                m_bufs.append(mb)
            pu32s, pv32s = [], []
            for k in range(NSLAB):
                pu_t = pstat.tile([NPART, SLAB], F32, name=f"PU{k}", tag=f"PU{k}")
                pv_t = pstat.tile([NPART, SLAB], F32, name=f"PV{k}", tag=f"PV{k}")
                pu32s.append(pu_t)
                pv32s.append(pv_t)

            for k in range(NSLAB):
                sl = slice(k * SLAB, (k + 1) * SLAB)
                mb = m_bufs[k]
                nc.sync.dma_start(mb[:, S_ONE * SLAB:(S_ONE + 1) * SLAB], st16_d.ap()[0, :, sl])
                nc.sync.dma_start(
                    mb[:, S_PUA * SLAB:(S_PVB + 1) * SLAB],
                    st16_d.ap()[1:5].rearrange("a p c -> p a c")[:, :, sl],
                )
                nc.sync.dma_start(pu32s[k][:], st32_d.ap()[0, :, sl])
                nc.sync.dma_start(pv32s[k][:], st32_d.ap()[1, :, sl])

            ps = pps.tile([C1, SPC * NCH], F32)
            delta_t = pstat.tile([NPART, 1], F32, name="delta", tag="delta")
            nc.vector.memset(delta_t[:], DELTA)

            loop_cm = tc.For_i(0, reps, 1) if reps > 1 else contextlib.nullcontext()
            with loop_cm:
              for s in range(SPC):
                for k in range(NSLAB):
                    sl = slice(k * SLAB, (k + 1) * SLAB)
                    mb = m_bufs[k]
                    # --- loads (one contiguous-run DMA per plane) ---
                    feat_eng = nc.gpsimd if feat_q == "gpsimd" else nc.sync
                    sub = SLAB // dma_split
                    for a in range(8):
                        for d in range(dma_split):
                            feat_eng.dma_start(
                                mb[:, (S_Q + a) * SLAB + d * sub:(S_Q + a) * SLAB + (d + 1) * sub],
                                feat_d.ap()[s, a, :, k * SLAB + d * sub:k * SLAB + (d + 1) * sub],
                            )
                    cat_t = pwork.tile([NPART, SLAB], MBF16, name=f"cat_{s}_{k}", tag="cat")
                    x0 = pwork.tile([NPART, SLAB], MBF16, name=f"x0_{s}_{k}", tag="x0")
                    x1 = pwork.tile([NPART, SLAB], MBF16, name=f"x1_{s}_{k}", tag="x1")
                    nc.sync.dma_start(cat_t[:], cat_d.ap()[s, :, sl])
                    nc.sync.dma_start(x0[:], xy_d.ap()[s, 0, :, sl])
                    nc.sync.dma_start(x1[:], xy_d.ap()[s, 1, :, sl])

                    # --- one-hot masks (DVE tensor_scalar is_equal, bf16) ---
                    oh = pwork.tile([NPART, C1 * SLAB], MBF16, name=f"oh_{s}_{k}", tag="oh")
                    cmp_eng = nc.gpsimd if cmp_gps else nc.vector
                    for c in range(1, CLASSES):
                        cmp_eng.tensor_scalar(
                            oh[:, (c - 1) * SLAB:c * SLAB], cat_t[:],
                            float(c), None, op0=AOT.is_equal,
                        )

                    # --- per-pixel direction weights ---
                    dxh2 = mb[:, S_DXH2 * SLAB:(S_DXH2 + 1) * SLAB]
                    dyh2 = mb[:, S_DYH2 * SLAB:(S_DYH2 + 1) * SLAB]
                    mm_ = mb[:, S_M * SLAB:(S_M + 1) * SLAB]
                    if dve_v2:
                        # n2 = x0^2 + x1^2 (ACT squares f32, one bf16 2x add);
                        # rr = 1/sqrt(n2+delta) in bf16; dxh = x0*rr, dyh = x1*rr
                        # (bf16 2x); dxh2/dyh2 via ACT Square straight into the
                        # moving slots; m = dxh*dyh (bf16 2x). No sxy, no fp32
                        # 1x multiplies on DVE.
                        sx = ptmp.tile([NPART, SLAB], MBF16, name=f"sx_{s}_{k}", tag="sx")
                        sy = ptmp.tile([NPART, SLAB], MBF16, name=f"sy_{s}_{k}", tag="sy")
                        nc.scalar.square(sx[:], x0[:])
                        nc.scalar.square(sy[:], x1[:])
                        n2 = ptmp.tile([NPART, SLAB], MBF16, name=f"n2_{s}_{k}", tag="n2")
                        nc.vector.tensor_tensor(n2[:], sx[:], sy[:], op=AOT.add)
                        rr = ptmp.tile([NPART, SLAB], MBF16, name=f"rr_{s}_{k}", tag="rr")
                        nc.scalar.activation(rr[:], n2[:], ACTF.Abs_reciprocal_sqrt, bias=delta_t[:])
                        dxh = ptmp.tile([NPART, SLAB], MBF16, name=f"dxh_{s}_{k}", tag="dxh")
                        dyh = ptmp.tile([NPART, SLAB], MBF16, name=f"dyh_{s}_{k}", tag="dyh")
                        nc.vector.tensor_tensor(dxh[:], x0[:], rr[:], op=AOT.mult)
                        nc.vector.tensor_tensor(dyh[:], x1[:], rr[:], op=AOT.mult)
                        nc.scalar.square(dxh2, dxh[:])
                        nc.scalar.square(dyh2, dyh[:])
                        nc.vector.tensor_tensor(mm_, dxh[:], dyh[:], op=AOT.mult)
                    else:
                        sx = ptmp.tile([NPART, SLAB], F32, name=f"sx_{s}_{k}", tag="sx")
                        sy = ptmp.tile([NPART, SLAB], F32, name=f"sy_{s}_{k}", tag="sy")
                        nc.scalar.square(sx[:], x0[:])
                        nc.scalar.square(sy[:], x1[:])
                        sxy = ptmp.tile([NPART, SLAB], F32, name=f"sxy_{s}_{k}", tag="sxy")
                        n2 = ptmp.tile([NPART, SLAB], F32, name=f"n2_{s}_{k}", tag="n2")
                        nc.vector.tensor_tensor(sxy[:], x0[:], x1[:], op=AOT.mult)
                        nc.vector.tensor_tensor(n2[:], sx[:], sy[:], op=AOT.add)
                        rr = ptmp.tile([NPART, SLAB], F32, name=f"rr_{s}_{k}", tag="rr")
                        r2 = ptmp.tile([NPART, SLAB], F32, name=f"r2_{s}_{k}", tag="r2")
                        nc.scalar.activation(rr[:], n2[:], ACTF.Abs_reciprocal_sqrt, bias=delta_t[:])
                        nc.scalar.square(r2[:], rr[:])
                        nc.vector.tensor_tensor(dxh2, sx[:], r2[:], op=AOT.mult)
                        nc.vector.tensor_tensor(dyh2, sy[:], r2[:], op=AOT.mult)
                        nc.vector.tensor_tensor(mm_, sxy[:], r2[:], op=AOT.mult)
                    nc.vector.tensor_tensor(
                        mb[:, S_T1 * SLAB:(S_T1 + 1) * SLAB], dxh2, pu32s[k][:], op=AOT.mult)
                    nc.vector.tensor_tensor(
                        mb[:, S_T2 * SLAB:(S_T2 + 1) * SLAB], mm_, pv32s[k][:], op=AOT.mult)
                    nc.vector.tensor_tensor(
                        mb[:, S_T3 * SLAB:(S_T3 + 1) * SLAB], mm_, pu32s[k][:], op=AOT.mult)
                    nc.vector.tensor_tensor(
                        mb[:, S_T4 * SLAB:(S_T4 + 1) * SLAB], dyh2, pv32s[k][:], op=AOT.mult)

                    # --- PE segmented-sum stream: one matmul per 128-px chunk ---
                    oh_r = oh[:].rearrange("p (c s) -> p c s", c=C1)
                    mv_r = mb[:].rearrange("p (c s) -> p c s", c=NCH)
                    for j in range(SLAB):
                        nc.tensor.matmul(
                            ps[:, s * NCH:(s + 1) * NCH],
                            oh_r[:, :, j],
                            mv_r[:, :, j],
                            start=(k == 0 and j == 0),
                            stop=(k == NSLAB - 1 and j == SLAB - 1),
                            skip_group_check=True,
                        )

            outs = ptmp.tile([C1, SPC * NCH], F32)
            nc.vector.tensor_copy(outs[:], ps[:])
            nc.sync.dma_start(sums_d.ap()[:, :], outs[:])

    nc.compile()
    _NC_CACHE[key] = nc
    return nc


def _host_prep(inputs):
    """Build per-core input maps (bf16 planes in [128, 2400] partition-major layout)."""
    cat = np.asarray(inputs["cat_mask"])
    quat = np.asarray(inputs["quaternion"], dtype=np.float32)
    scales = np.asarray(inputs["scales"], dtype=np.float32)
    xy = np.asarray(inputs["xy"], dtype=np.float32)
    z = np.asarray(inputs["z"], dtype=np.float32)

    st16, st32 = _build_static()

    feat = np.concatenate(
        [quat.reshape(B, 4, H * W), scales.reshape(B, 3, H * W),
         z.reshape(B, 1, H * W)], axis=1,
    ).reshape(B, 8, NPART, COLS).astype(BF16)
    cat16 = cat.reshape(B, NPART, COLS).astype(BF16)
    xy16 = xy.reshape(B, 2, NPART, COLS).astype(BF16)

    in_maps = []
    for i in range(NCORES):
        sl = slice(i * SPC, (i + 1) * SPC)
        in_maps.append({
            "feat": np.ascontiguousarray(feat[sl]),
            "cat": np.ascontiguousarray(cat16[sl]),
            "xy": np.ascontiguousarray(xy16[sl]),
            "st16": st16,
            "st32": st32,
        })
    return in_maps


def _host_finish(sums_all, intrinsics):
    """sums_all: [B, C1, NCH] float64. Returns [C1, B, 26] float32."""
    S = sums_all
    cnt = S[..., S_ONE]
    denom = np.maximum(cnt, 1.0)
    q_agg = S[..., S_Q:S_Q + 4] / denom[..., None]
    s_agg = S[..., S_S:S_S + 3] / denom[..., None]
    z_agg = S[..., S_Z] / denom

    Axx = cnt - S[..., S_DXH2]
    Ayy = cnt - S[..., S_DYH2]
    Axy = -S[..., S_M]
    Spu = S[..., S_PUA] + S[..., S_PUB]
    Spv = S[..., S_PVA] + S[..., S_PVB]
    rx = Spu - S[..., S_T1] - S[..., S_T2]
    ry = Spv - S[..., S_T3] - S[..., S_T4]

    A = np.empty(S.shape[:2] + (2, 2))
    A[..., 0, 0] = Axx + EPS
    A[..., 0, 1] = Axy
    A[..., 1, 0] = Axy
    A[..., 1, 1] = Ayy + EPS
    rhs = np.stack([rx, ry], axis=-1)
    center = np.linalg.solve(A, rhs[..., None])[..., 0]  # [B, C1, 2]

    qn = q_agg / (np.linalg.norm(q_agg, axis=-1, keepdims=True) + 1e-8)
    w, x, y, zz = qn[..., 0], qn[..., 1], qn[..., 2], qn[..., 3]
    R = np.stack([
        1 - 2 * (y * y + zz * zz), 2 * (x * y - w * zz), 2 * (x * zz + w * y),
        2 * (x * y + w * zz), 1 - 2 * (x * x + zz * zz), 2 * (y * zz - w * x),
        2 * (x * zz - w * y), 2 * (y * zz + w * x), 1 - 2 * (x * x + y * y),
    ], axis=-1).reshape(S.shape[:2] + (3, 3))

    zval = np.exp(z_agg)
    Kinv = np.linalg.inv(np.asarray(intrinsics, dtype=np.float64))
    homog = np.concatenate([center, np.ones(S.shape[:2] + (1,))], axis=-1)
    t = zval[..., None] * np.einsum("ij,bcj->bci", Kinv, homog)

    RT = np.zeros(S.shape[:2] + (4, 4))
    RT[..., :3, :3] = R
    RT[..., :3, 3] = t
    RT[..., 3, 3] = 1.0

    out = np.concatenate(
        [q_agg, s_agg, z_agg[..., None], center, RT.reshape(S.shape[:2] + (16,))],
        axis=-1,
    )  # [B, C1, 26]
    return np.transpose(out, (1, 0, 2)).astype(np.float32)


def kernel(**inputs):
    from concourse.bass_utils import run_bass_kernel_spmd

    nc = _build_nc()
    in_maps = _host_prep(inputs)
    res = run_bass_kernel_spmd(nc, in_maps, core_ids=list(range(NCORES)))
    sums_all = np.empty((B, C1, NCH), dtype=np.float64)
    for i in range(NCORES):
        s = res.results[i]["sums"].astype(np.float64)  # [C1, SPC*NCH]
        for j in range(SPC):
            sums_all[i * SPC + j] = s[:, j * NCH:(j + 1) * NCH]
    return _host_finish(sums_all, inputs["intrinsics"])



# revision 7
# speedup vs baseline: 5.8794x; 5.8794x over previous
"""Trainium2 Bass kernel for nn_AggregationLayer (per-class masked reductions + Hough voting).

Strategy (8 NeuronCores, data-parallel over batch: 2 samples/core):
  Per (class c in 1..6, sample b) the device computes 17 masked sums
      S_c[x] = sum_p [cat_p == c] * x_p
  over the 307200 pixels of a sample, for channels
      x in {1, r, t, q0..q3, s0..s2, z, dxh2, m, dxh2*r, m*r, dxh2*t, m*t}
  where (dxh2, m) = (x0^2, x0*x1) / (n2 + delta) are the Hough direction
  terms (dyh2 is recovered from dxh2 via dx^2+dy^2=1), r is the partition
  index and t = chunk//20 a per-column ramp.

  Pixels are laid out column-major: pixel p = chunk*128 + r, so
  pu = p%640 = 128*(chunk%5) + r and pv = p//640 = 4*(chunk//20) + (chunk%20)//5.
  The per-chunk offsets are recovered on the host from the chunk slot j
  (= chunk%20) of each partial sum, and the remaining position dependence is
  exactly covered by the r and t channels.

  The segmented reduction runs on the TensorEngine as 120 matmuls/sample:
  each matmul contracts ONE group of 20 chunks at once. Stationary = the
  group's one-hot columns [128, 20*6], moving = the group's channel values
  [128, 17*20], PSUM [120, 340] accumulates across groups. Only the 20
  "diagonal" [6, 17] blocks (stationary chunk-slot j == moving chunk-slot j)
  are used; off-diagonal products are ignored. This replaces the naive
  one-matmul-per-chunk stream (4800 instruction-bound tiny matmuls) with
  240 large ones.

  The host does only the tiny [6, B] finalization: diagonal extraction with
  per-slot position coefficients, 2x2 solve for the Hough center,
  quaternion -> rotation matrix, intrinsics backprojection, packing into the
  [C-1, B, 26] output.
"""

import numpy as np
import ml_dtypes

B, H, W = 16, 480, 640
CLASSES = 7
C1 = CLASSES - 1
NCORES = 8
SPC = B // NCORES          # samples per core
NPART = 128
COLS = (H * W) // NPART    # 2400 chunks per sample
SLAB = 800
NSLAB = COLS // SLAB       # 3
G = 20                     # chunk-slots per matmul group (multiple of 5)
NGRP = SLAB // G           # 40 groups per slab
NCH = 17                   # moving channels
NMOV = NCH * G             # 340 moving cols per matmul
NSTAT = C1 * G             # 120 stationary cols per matmul
DELTA = 1e-12              # guard for 1/(n2 + DELTA)
EPS = 1e-6                 # matches reference

BF16 = ml_dtypes.bfloat16

# moving-channel slots
S_ONE, S_R, S_T = 0, 1, 2
S_Q = 3                    # 3..6
S_S = 7                    # 7..9
S_Z = 10
S_DXH2, S_M = 11, 12
S_DXR, S_MR = 13, 14
S_DXT, S_MT = 15, 16

_NC_CACHE = {}
_STATIC_CACHE = {}


def _build_static():
    if "st" in _STATIC_CACHE:
        return _STATIC_CACHE["st"]
    ones = np.ones((NPART, COLS), dtype=np.float64)
    rvec = np.broadcast_to(np.arange(NPART, dtype=np.float64)[:, None], (NPART, COLS))
    tvec = np.broadcast_to((np.arange(COLS, dtype=np.float64) // G)[None, :], (NPART, COLS))
    st = np.stack([ones, rvec, tvec]).astype(BF16)       # [3, 128, 2400]
    rcol = np.arange(NPART, dtype=np.float32)[:, None]   # [128, 1]
    _STATIC_CACHE["st"] = (st, rcol)
    return st, rcol


def _build_nc(reps=1, cmp_split=3):
    """Build + compile the SPMD Bass program. reps > 1 wraps the whole
    pipeline in a hardware For loop (used only for benchmarking).
    cmp_split: number of the 6 one-hot compares run on DVE (rest on GPSIMD)."""
    key = (reps, cmp_split, SLAB, G)
    if key in _NC_CACHE:
        return _NC_CACHE[key]
    import contextlib
    import concourse.bacc as bacc
    import concourse.mybir as mybir
    import concourse.tile as tile

    F32, MBF16 = mybir.dt.float32, mybir.dt.bfloat16
    AOT = mybir.AluOpType
    ACTF = mybir.ActivationFunctionType

    nc = bacc.Bacc("TRN2", target_bir_lowering=False, debug=False)
    feat_d = nc.dram_tensor("feat", [SPC, 8, NPART, COLS], MBF16, kind="ExternalInput")
    cxy_d = nc.dram_tensor("cxy", [SPC, 3, NPART, COLS], MBF16, kind="ExternalInput")
    st_d = nc.dram_tensor("st", [3, NPART, COLS], MBF16, kind="ExternalInput")
    rcol_d = nc.dram_tensor("rcol", [NPART, 1], F32, kind="ExternalInput")
    sums_d = nc.dram_tensor("sums", [NSTAT, SPC * NMOV], F32, kind="ExternalOutput")

    with tile.TileContext(nc) as tc:
        with (
            tc.tile_pool(name="mov", bufs=1) as pmov,
            tc.tile_pool(name="stat", bufs=1) as pstat,
            tc.tile_pool(name="work", bufs=3) as pwork,
            tc.tile_pool(name="tmp", bufs=3) as ptmp,
            tc.tile_pool(name="psum", bufs=1, space="PSUM") as pps,
        ):
            # persistent moving buffers, one per slab phase; static channel
            # slots (ones/r/t) are written once per physical buffer and
            # survive the per-sample rewrites of slots 3..16
            m_bufs = []
            for k in range(NSLAB):
                mb = pmov.tile([NPART, NCH * SLAB], MBF16, name=f"Mbuf{k}", tag=f"Mbuf{k}")
                m_bufs.append(mb)
                sl = slice(k * SLAB, (k + 1) * SLAB)
                nc.sync.dma_start(
                    mb[:, 0:3 * SLAB],
                    st_d.ap().rearrange("a p c -> p a c")[:, :, sl],
                )
            rcol_t = pstat.tile([NPART, 1], F32, name="rcol", tag="rcol")
            nc.sync.dma_start(rcol_t[:], rcol_d.ap()[:, :])
            delta_t = pstat.tile([NPART, 1], F32, name="delta", tag="delta")
            nc.vector.memset(delta_t[:], DELTA)

            ps_list = [pps.tile([NSTAT, NMOV], F32, name=f"ps{s}", tag=f"ps{s}")
                       for s in range(SPC)]
            outs = pstat.tile([NSTAT, SPC * NMOV], F32, name="outs", tag="outs")

            loop_cm = tc.For_i(0, reps, 1) if reps > 1 else contextlib.nullcontext()
            with loop_cm:
              for s in range(SPC):
                for k in range(NSLAB):
                    sl = slice(k * SLAB, (k + 1) * SLAB)
                    mb = m_bufs[k]
                    ps = ps_list[s]

                    cxy = pwork.tile([NPART, 3 * SLAB], MBF16, name=f"cxy_{s}_{k}", tag="cxy")
                    nc.sync.dma_start(
                        cxy[:], cxy_d.ap()[s].rearrange("a p c -> p a c")[:, :, sl])
                    nc.sync.dma_start(
                        mb[:, S_Q * SLAB:(S_Z + 1) * SLAB],
                        feat_d.ap()[s].rearrange("a p c -> p a c")[:, :, sl])
                    cat_t = cxy[:, 0:SLAB]
                    x0 = cxy[:, SLAB:2 * SLAB]
                    x1 = cxy[:, 2 * SLAB:3 * SLAB]

                    # one-hot masks, interleaved layout: col = 6*chunk + (c-1)
                    # so each matmul group's stationary slice is contiguous
                    # (walrus requires a single-free-dim weights AP)
                    oh = pwork.tile([NPART, C1 * SLAB], MBF16, name=f"oh_{s}_{k}", tag="oh")
                    oh_i = oh[:].rearrange("p (q c) -> p q c", c=C1)
                    for c in range(1, CLASSES):
                        eng = nc.vector if (c - 1) < cmp_split else nc.gpsimd
                        eng.tensor_scalar(
                            oh_i[:, :, c - 1], cat_t,
                            float(c), None, op0=AOT.is_equal,
                        )

                    # direction weights via rr = 1/sqrt(n2+d):
                    # dxh2 = (x0*rr)^2, m = (x0*rr)*(x1*rr)
                    sx = ptmp.tile([NPART, SLAB], MBF16, name=f"sx_{s}_{k}", tag="sx")
                    sy = ptmp.tile([NPART, SLAB], MBF16, name=f"sy_{s}_{k}", tag="sy")
                    nc.scalar.square(sx[:], x0)
                    nc.scalar.square(sy[:], x1)
                    n2 = ptmp.tile([NPART, SLAB], MBF16, name=f"n2_{s}_{k}", tag="n2")
                    nc.vector.tensor_tensor(n2[:], sx[:], sy[:], op=AOT.add)
                    rr = ptmp.tile([NPART, SLAB], MBF16, name=f"rr_{s}_{k}", tag="rr")
                    nc.scalar.activation(rr[:], n2[:], ACTF.Abs_reciprocal_sqrt, bias=delta_t[:])
                    dxh = ptmp.tile([NPART, SLAB], MBF16, name=f"dxh_{s}_{k}", tag="dxh")
                    dyh = ptmp.tile([NPART, SLAB], MBF16, name=f"dyh_{s}_{k}", tag="dyh")
                    nc.vector.tensor_tensor(dxh[:], x0, rr[:], op=AOT.mult)
                    nc.vector.tensor_tensor(dyh[:], x1, rr[:], op=AOT.mult)
                    dxh2 = mb[:, S_DXH2 * SLAB:(S_DXH2 + 1) * SLAB]
                    mm_ = mb[:, S_M * SLAB:(S_M + 1) * SLAB]
                    nc.scalar.square(dxh2, dxh[:])
                    nc.vector.tensor_tensor(mm_, dxh[:], dyh[:], op=AOT.mult)
                    # r- and t-weighted copies straight into the moving slots
                    nc.scalar.activation(
                        mb[:, S_DXR * SLAB:(S_DXR + 1) * SLAB], dxh2,
                        ACTF.Copy, scale=rcol_t[:])
                    nc.scalar.activation(
                        mb[:, S_MR * SLAB:(S_MR + 1) * SLAB], mm_,
                        ACTF.Copy, scale=rcol_t[:])
                    tvec = mb[:, S_T * SLAB:(S_T + 1) * SLAB]
                    nc.vector.tensor_tensor(
                        mb[:, S_DXT * SLAB:(S_DXT + 1) * SLAB], dxh2, tvec, op=AOT.mult)
                    nc.vector.tensor_tensor(
                        mb[:, S_MT * SLAB:(S_MT + 1) * SLAB], mm_, tvec, op=AOT.mult)

                    # PE segmented-sum: one matmul per 20-chunk group
                    mv_r = mb[:].rearrange("p (a q) -> p a q", a=NCH)  # [128, 17, 800]
                    for g in range(NGRP):
                        nc.tensor.matmul(
                            ps[:, :],
                            oh[:, g * NSTAT:(g + 1) * NSTAT],
                            mv_r[:, :, g * G:(g + 1) * G],
                            start=(k == 0 and g == 0),
                            stop=(k == NSLAB - 1 and g == NGRP - 1),
                            skip_group_check=True,
                        )

            for s in range(SPC):
                nc.vector.tensor_copy(outs[:, s * NMOV:(s + 1) * NMOV], ps_list[s][:, :])
            nc.sync.dma_start(sums_d.ap()[:, :], outs[:])

    nc.compile()
    _NC_CACHE[key] = nc
    return nc


def _host_prep(inputs):
    """Per-core input maps. Pixel layout is column-major: plane[r, chunk]
    holds pixel p = chunk*128 + r, so each 128-pixel chunk is one column."""
    cat = np.asarray(inputs["cat_mask"])
    quat = np.asarray(inputs["quaternion"], dtype=np.float32)
    scales = np.asarray(inputs["scales"], dtype=np.float32)
    xy = np.asarray(inputs["xy"], dtype=np.float32)
    z = np.asarray(inputs["z"], dtype=np.float32)

    st, rcol = _build_static()

    def planes(a):
        # [B, A, H*W] -> [B, A, 128, 2400] column-major pixels
        return a.reshape(a.shape[0], a.shape[1], COLS, NPART).swapaxes(2, 3)

    feat = planes(np.concatenate(
        [quat.reshape(B, 4, H * W), scales.reshape(B, 3, H * W),
         z.reshape(B, 1, H * W)], axis=1)).astype(BF16)
    cxy = planes(np.stack(
        [cat.reshape(B, H * W).astype(np.float32),
         xy.reshape(B, 2, H * W)[:, 0], xy.reshape(B, 2, H * W)[:, 1]],
        axis=1)).astype(BF16)

    in_maps = []
    for i in range(NCORES):
        sl = slice(i * SPC, (i + 1) * SPC)
        in_maps.append({
            "feat": np.ascontiguousarray(feat[sl]),
            "cxy": np.ascontiguousarray(cxy[sl]),
            "st": st,
            "rcol": rcol,
        })
    return in_maps


def _host_finish(sums_all, intrinsics):
    """sums_all: [B, NSTAT, NMOV] float64 (PSUM dumps). Returns [C1, B, 26] f32."""
    A = sums_all.reshape(B, G, C1, NCH, G)
    r = np.arange(G)
    Dd = A[:, r, :, :, r]                    # [G, B, C1, NCH] diagonal blocks
    off = (128.0 * (r % 5))[:, None, None]
    fl = (r // 5).astype(np.float64)[:, None, None]

    S = Dd.sum(axis=0)                       # [B, C1, NCH] plain sums
    Su1 = (off * Dd[..., S_ONE] + Dd[..., S_R]).sum(axis=0)
    Sv1 = (4.0 * Dd[..., S_T] + fl * Dd[..., S_ONE]).sum(axis=0)
    Sudx = (off * Dd[..., S_DXH2] + Dd[..., S_DXR]).sum(axis=0)
    Svdx = (4.0 * Dd[..., S_DXT] + fl * Dd[..., S_DXH2]).sum(axis=0)
    Sum_ = (off * Dd[..., S_M] + Dd[..., S_MR]).sum(axis=0)
    Svm = (4.0 * Dd[..., S_MT] + fl * Dd[..., S_M]).sum(axis=0)

    cnt = S[..., S_ONE]
    denom = np.maximum(cnt, 1.0)
    q_agg = S[..., S_Q:S_Q + 4] / denom[..., None]
    s_agg = S[..., S_S:S_S + 3] / denom[..., None]
    z_agg = S[..., S_Z] / denom

    Axx = cnt - S[..., S_DXH2] + EPS
    Ayy = S[..., S_DXH2] + EPS
    Axy = -S[..., S_M]
    rx = Su1 - Sudx - Svm
    ry = Svdx - Sum_

    det = Axx * Ayy - Axy * Axy
    cx = (Ayy * rx - Axy * ry) / det
    cy = (Axx * ry - Axy * rx) / det
    center = np.stack([cx, cy], axis=-1)     # [B, C1, 2]

    qn = q_agg / (np.linalg.norm(q_agg, axis=-1, keepdims=True) + 1e-8)
    w, x, y, zz = qn[..., 0], qn[..., 1], qn[..., 2], qn[..., 3]
    R = np.stack([
        1 - 2 * (y * y + zz * zz), 2 * (x * y - w * zz), 2 * (x * zz + w * y),
        2 * (x * y + w * zz), 1 - 2 * (x * x + zz * zz), 2 * (y * zz - w * x),
        2 * (x * zz - w * y), 2 * (y * zz + w * x), 1 - 2 * (x * x + y * y),
    ], axis=-1).reshape(cnt.shape + (3, 3))

    zval = np.exp(z_agg)
    Kinv = np.linalg.inv(np.asarray(intrinsics, dtype=np.float64))
    homog = np.concatenate([center, np.ones(cnt.shape + (1,))], axis=-1)
    t = zval[..., None] * np.einsum("ij,bcj->bci", Kinv, homog)

    RT = np.zeros(cnt.shape + (4, 4))
    RT[..., :3, :3] = R
    RT[..., :3, 3] = t
    RT[..., 3, 3] = 1.0

    out = np.concatenate(
        [q_agg, s_agg, z_agg[..., None], center, RT.reshape(cnt.shape + (16,))],
        axis=-1,
    )  # [B, C1, 26]
    return np.transpose(out, (1, 0, 2)).astype(np.float32)


def kernel(**inputs):
    from concourse.bass_utils import run_bass_kernel_spmd

    nc = _build_nc()
    in_maps = _host_prep(inputs)
    res = run_bass_kernel_spmd(nc, in_maps, core_ids=list(range(NCORES)))
    sums_all = np.empty((B, NSTAT, NMOV), dtype=np.float64)
    for i in range(NCORES):
        s = res.results[i]["sums"].astype(np.float64)  # [NSTAT, SPC*NMOV]
        for j in range(SPC):
            sums_all[i * SPC + j] = s[:, j * NMOV:(j + 1) * NMOV]
    return _host_finish(sums_all, inputs["intrinsics"])
